# revision 15
# baseline (speedup 1.0000x reference)
"""GroupedQueryAttention Trainium2 Bass kernel (v2).

Sharding: 8 cores = (B=2) x (G=4 KV groups). Each core computes, for its
(batch b, kv-group g): the 4 query heads' Q/K/V projections, causal flash
attention, and a partial output projection Y^T_g (bf16). Host sums the 4
partials per batch and adds an adjusted bias (bo + bv-term folded in).

Key structure (all transposed: token dim T on the free axis):
  xT[d, t]     uploaded pre-transposed from host (bf16) - no PE transposes
  Q^T, K^T     from projection matmuls (W chunk stationary, xT moving)
  V^T -> V     PE transpose per 128-block, staged in the st PSUM slots
  S^T[s, t]  = (K^T s-block).T @ Q^T        (one 128-wide matmul per s-block)
  P^T        = exp(scale * S^T + mask)      (ACT, PSUM -> SBUF, bf16)
  O^T[dh, t] += (V s-block).T @ P^T         (PSUM accumulation over s-blocks)
  rowsum     += ones.T @ P^T                (PSUM accumulation, M=1)
  Y^T[dm, t] = sum_c (Wo chunk).T @ O^T_c   (per 128-row dm block, bf16 out)

The V bias never enters the kernel: O = (P@(V0+1*bv))/rowsum = P@V0/rowsum
+ bv, and the constant bv contribution to Y is folded into bo on the host.

oproj(tau-1) m-blocks are interleaved into flash(tau) as PE filler so the
S->exp->PV dependency chain's bubbles are absorbed by dense matmul work.

Normalize chain: rs copy (scalar) -> reciprocal_approx_fast [1,512] (DVE)
-> partition_broadcast (gpsimd) -> mul (DVE). Avoids the 3.3us exact
reciprocal.
"""

import sys

sys.path.insert(0, "/opt/trn_rl_repo")

from contextlib import ExitStack

import ml_dtypes
import numpy as np

import concourse.bass as bass  # noqa: F401
import concourse.tile as tile
from concourse import bacc, mybir
from concourse.bass_utils import run_bass_kernel_spmd

F32 = mybir.dt.float32
BF16 = mybir.dt.bfloat16
AF = mybir.ActivationFunctionType

D = 2048          # model dim
T = 2048          # tokens
DH = 128          # head dim
G = 4             # kv groups
HPG = 4           # query heads per group
QC = HPG * DH     # query cols per group = 512
ND = D // 128     # 16 contraction chunks
NTAU = 4          # t tiles of 512
TW = 512          # t tile width
SCALE = DH ** -0.5
NEG = -1e30

TRACE = False
TRACE_KW = {}
LAST_RESULTS = None

_CACHE = {}


def _body(ctx, tc, xT, wq, wk, wv, wo, bq, bk, maskTd, identd, yT):
    nc = tc.nc

    # PSUM: acc(2) + st(3, shared with V-transpose staging) + ot(2) + rs(1)
    psacc = ctx.enter_context(tc.tile_pool(name="psacc", bufs=2, space="PSUM"))
    psst = ctx.enter_context(tc.tile_pool(name="psst", bufs=3, space="PSUM"))
    psot = ctx.enter_context(tc.tile_pool(name="psot", bufs=2, space="PSUM"))
    psrs = ctx.enter_context(tc.tile_pool(name="psrs", bufs=1, space="PSUM"))

    consts = ctx.enter_context(tc.tile_pool(name="consts", bufs=1))
    qkv = ctx.enter_context(tc.tile_pool(name="qkv", bufs=1))
    xtp = ctx.enter_context(tc.tile_pool(name="xtp", bufs=ND))
    wkp = ctx.enter_context(tc.tile_pool(name="wkp", bufs=ND))
    wvp = ctx.enter_context(tc.tile_pool(name="wvp", bufs=ND))
    wqp = ctx.enter_context(tc.tile_pool(name="wqp", bufs=ND))
    wop = ctx.enter_context(tc.tile_pool(name="wop", bufs=1))
    vts = ctx.enter_context(tc.tile_pool(name="vstage", bufs=2))
    ptp = ctx.enter_context(tc.tile_pool(name="ptp", bufs=3))
    nrm = ctx.enter_context(tc.tile_pool(name="norm", bufs=2))
    otp_pool = ctx.enter_context(tc.tile_pool(name="otsb", bufs=1))
    yb = ctx.enter_context(tc.tile_pool(name="ybounce", bufs=3))

    # ---- constants + weights (SWDGE on gpsimd, ordered by first use)
    maskt = consts.tile([128, 128], F32, tag="maskt")
    nc.gpsimd.dma_start(maskt, maskTd)
    bqt = consts.tile([128, 4], F32, tag="bqt")
    nc.gpsimd.dma_start(bqt, bq.rearrange("(c p) -> p c", p=128))
    bkt = consts.tile([128, 1], F32, tag="bkt")
    nc.gpsimd.dma_start(bkt, bk.rearrange("(c p) -> p c", p=128))
    ident = consts.tile([128, 128], BF16, tag="ident")
    nc.gpsimd.dma_start(ident, identd)
    ones_col = consts.tile([128, 1], BF16, tag="ones_col")
    nc.vector.memset(ones_col, 1.0)

    # ---- weights + x: 3 DMA queues (sync, scalar HWDGE; gpsimd SWDGE).
    # Priority order: wk first (K(sg0) gate), then wave0 = interleaved
    # {x(sg0,d), wq(d), wv(d)} so flash(tau=0) can start ~20us in, then the
    # remaining x column-waves, wo last (first needed ~40us later).
    xts = [xtp.tile([128, T], BF16, tag="xt", name=f"xt{d}") for d in range(ND)]
    wkts = [wkp.tile([128, DH], BF16, tag="wk", name=f"wk{d}") for d in range(ND)]
    wvts = [wvp.tile([128, DH], BF16, tag="wv", name=f"wv{d}") for d in range(ND)]
    wqts = [wqp.tile([128, QC], BF16, tag="wq", name=f"wq{d}") for d in range(ND)]
    wot = [wop.tile([128, D], BF16, tag=f"wo{c}", name=f"wo{c}") for c in range(HPG)]

    for d in range(ND):
        nc.gpsimd.dma_start(wkts[d], wk[d * 128:(d + 1) * 128, :])
    qlist = [nc.sync, nc.scalar, nc.gpsimd]
    qi = 0

    def q_next():
        nonlocal qi
        eng = qlist[qi % 3]
        qi += 1
        return eng

    for d in range(ND):
        q_next().dma_start(xts[d][:, 0:TW], xT[d * 128:(d + 1) * 128, 0:TW])
        q_next().dma_start(wqts[d], wq[d * 128:(d + 1) * 128, :])
        q_next().dma_start(wvts[d], wv[d * 128:(d + 1) * 128, :])
    for sg in range(1, NTAU):
        for d in range(ND):
            q_next().dma_start(
                xts[d][:, sg * TW:(sg + 1) * TW],
                xT[d * 128:(d + 1) * 128, sg * TW:(sg + 1) * TW])
    for c in range(HPG):
        nc.gpsimd.dma_start(wot[c], wo[c * 128:(c + 1) * 128, :])

    qt = [qkv.tile([128, T], BF16, tag=f"qt{j}", name=f"qt{j}") for j in range(HPG)]
    kt = qkv.tile([128, T], BF16, tag="kt")
    vv = qkv.tile([128, ND, 128], BF16, tag="vv")  # [s%128, s_block, dh]

    # ---- K + V projection for one sg column block
    def kvproj(sg):
        ps = psacc.tile([128, TW], F32, tag="acc", name="psk")
        for d in range(ND):
            nc.tensor.matmul(ps, wkts[d], xts[d][:, sg * TW:(sg + 1) * TW],
                             start=(d == 0), stop=(d == ND - 1))
        nc.scalar.activation(kt[:, sg * TW:(sg + 1) * TW], ps, AF.Identity,
                             bias=bkt[:, 0:1])

        ps2 = psacc.tile([128, TW], F32, tag="acc", name="psv")
        for d in range(ND):
            nc.tensor.matmul(ps2, wvts[d], xts[d][:, sg * TW:(sg + 1) * TW],
                             start=(d == 0), stop=(d == ND - 1))
        vtt = vts.tile([128, TW], BF16, tag="vt")
        nc.scalar.copy(vtt, ps2)
        # V^T -> V native, staged in an st-tag PSUM slot (idle until flash)
        stg = psst.tile([128, TW], BF16, tag="st", name="vstg")
        for i in range(4):
            nc.tensor.transpose(stg[:, i * 128:(i + 1) * 128],
                                vtt[:, i * 128:(i + 1) * 128], ident)
        nc.vector.tensor_copy(vv[:, sg * 4:(sg + 1) * 4, :], stg)

    # ---- Q projection for one t-tile (4 head blocks)
    def qproj(tau):
        for cb in range(HPG):
            ps = psacc.tile([128, TW], F32, tag="acc", name="psq")
            for d in range(ND):
                nc.tensor.matmul(
                    ps, wqts[d][:, cb * 128:(cb + 1) * 128],
                    xts[d][:, tau * TW:(tau + 1) * TW],
                    start=(d == 0), stop=(d == ND - 1))
            nc.scalar.activation(qt[cb][:, tau * TW:(tau + 1) * TW], ps,
                                 AF.Identity, bias=bqt[:, cb:cb + 1])

    # ---- output projection m-block (4 matmuls + copy + store)
    ots = [otp_pool.tile([128, T], BF16, tag=f"ot{j}", name=f"ots{j}")
           for j in range(HPG)]

    def oproj_block(tau, m):
        yp = psacc.tile([128, TW], F32, tag="acc", name="yp")
        for c in range(HPG):
            nc.tensor.matmul(
                yp, wot[c][:, m * 128:(m + 1) * 128],
                ots[c][:, tau * TW:(tau + 1) * TW],
                start=(c == 0), stop=(c == HPG - 1))
        ys = yb.tile([128, TW], BF16, tag="y", name="ys")
        if m % 2 == 0:
            nc.vector.tensor_copy(ys, yp)
        else:
            nc.scalar.copy(ys, yp)
        nc.sync.dma_start(
            yT[m * 128:(m + 1) * 128, tau * TW:(tau + 1) * TW], ys)

    # ---- phase C: per-tau pipeline [K,V,Q, flash(+oproj filler)]
    for tau in range(NTAU):
        kvproj(tau)
        qproj(tau)
        nsb = 4 * tau + 4
        # filler units: oproj(tau-1) m-blocks, spread over this tau's steps
        filler = []
        if tau > 0:
            filler = [(tau - 1, m) for m in range(ND)]
        total_steps = HPG * nsb
        fill_every = max(1, total_steps // max(len(filler), 1))
        step = 0
        fi = 0

        for j in range(HPG):
            otp = psot.tile([128, TW], F32, tag="ot")
            rs = psrs.tile([1, TW], F32, tag="rs")
            pts = {}
            qslice = qt[j][:, tau * TW:(tau + 1) * TW]

            def consume(sb, last):
                pt_, lo_ = pts[sb]
                nc.tensor.matmul(otp[:, lo_:], vv[:, sb, :], pt_[:, lo_:],
                                 start=(sb == 0), stop=last)
                nc.tensor.matmul(rs[:, lo_:], ones_col, pt_[:, lo_:],
                                 start=(sb == 0), stop=last)

            for sb in range(nsb):
                di = sb - 4 * tau
                lo = di * 128 if di >= 0 else 0   # valid t-range start
                st = psst.tile([128, TW], F32, tag="st")
                nc.tensor.matmul(st[:, lo:], kt[:, sb * 128:(sb + 1) * 128],
                                 qslice[:, lo:], start=True, stop=True)
                if di >= 0:  # triangle mask on the first 128 valid columns
                    nc.vector.tensor_add(st[:, lo:lo + 128], st[:, lo:lo + 128],
                                         maskt)
                pt = ptp.tile([128, TW], BF16, tag="pt")
                nc.scalar.activation(pt[:, lo:], st[:, lo:], AF.Exp, scale=SCALE)
                pts[sb] = (pt, lo)
                # software-pipeline PE: PV/rowsum issue 2 s-blocks behind
                if sb >= 2:
                    consume(sb - 2, last=(sb - 2 == nsb - 1))
                    del pts[sb - 2]
                step += 1
                if fi < len(filler) and step % fill_every == 0:
                    oproj_block(*filler[fi])
                    fi += 1
            for sb in (nsb - 2, nsb - 1):
                if sb >= 0 and sb in pts:
                    consume(sb, last=(sb == nsb - 1))

            # normalize: O^T * (1/rowsum)
            rsb = nrm.tile([1, TW], F32, tag="rsb")
            nc.scalar.copy(rsb, rs)
            rc1 = nrm.tile([1, TW], F32, tag="rc1")
            nc.vector.reciprocal_approx_fast(rc1, rsb)
            rc = nrm.tile([128, TW], F32, tag="rc")
            nc.gpsimd.partition_broadcast(rc, rc1)
            nc.vector.tensor_mul(ots[j][:, tau * TW:(tau + 1) * TW], otp, rc)

        while fi < len(filler):
            oproj_block(*filler[fi])
            fi += 1
    for m in range(ND):
        oproj_block(NTAU - 1, m)


def _build_nc():
    if "nc" in _CACHE:
        return _CACHE["nc"]
    nc = bacc.Bacc("TRN2", target_bir_lowering=False, debug=False)
    xT = nc.dram_tensor("xT", [D, T], BF16, kind="ExternalInput").ap()
    wq = nc.dram_tensor("wq", [D, QC], BF16, kind="ExternalInput").ap()
    wk = nc.dram_tensor("wk", [D, DH], BF16, kind="ExternalInput").ap()
    wv = nc.dram_tensor("wv", [D, DH], BF16, kind="ExternalInput").ap()
    wo = nc.dram_tensor("wo", [QC, D], BF16, kind="ExternalInput").ap()
    bq = nc.dram_tensor("bq", [QC], F32, kind="ExternalInput").ap()
    bk = nc.dram_tensor("bk", [DH], F32, kind="ExternalInput").ap()
    maskTd = nc.dram_tensor("maskT", [128, 128], F32, kind="ExternalInput").ap()
    identd = nc.dram_tensor("ident", [128, 128], BF16, kind="ExternalInput").ap()
    yT = nc.dram_tensor("yT", [D, T], BF16, kind="ExternalOutput").ap()

    with tile.TileContext(nc) as tc, ExitStack() as ctx:
        _body(ctx, tc, xT, wq, wk, wv, wo, bq, bk, maskTd, identd, yT)
    nc.compile()
    _CACHE["nc"] = nc
    return nc


def _host_consts():
    p = np.arange(128)[:, None]
    f = np.arange(128)[None, :]
    maskT = np.where(f >= p, 0.0, NEG).astype(np.float32)
    ident = np.eye(128, dtype=ml_dtypes.bfloat16)
    return maskT, ident


def make_in_maps(x, Wq, bq, Wk, bk, Wv, bv, Wo, bo):
    maskT, ident = _host_consts()
    bf = lambda a: np.ascontiguousarray(a).astype(ml_dtypes.bfloat16)

    xTb = [bf(x[b].T) for b in range(2)]
    in_maps = []
    for c in range(8):
        b, g = divmod(c, G)
        in_maps.append({
            "xT": xTb[b],
            "wq": bf(Wq[:, g * QC:(g + 1) * QC]),
            "wk": bf(Wk[:, g * DH:(g + 1) * DH]),
            "wv": bf(Wv[:, g * DH:(g + 1) * DH]),
            "wo": bf(Wo[g * QC:(g + 1) * QC, :]),
            "bq": np.ascontiguousarray(bq[g * QC:(g + 1) * QC]),
            "bk": np.ascontiguousarray(bk[g * DH:(g + 1) * DH]),
            "maskT": maskT,
            "ident": ident,
        })
    return in_maps


def kernel(x, Wq, bq, Wk, bk, Wv, bv, Wo, bo):
    global LAST_RESULTS
    x = np.asarray(x, np.float32)
    Wq = np.asarray(Wq, np.float32)
    Wk = np.asarray(Wk, np.float32)
    Wv = np.asarray(Wv, np.float32)
    Wo = np.asarray(Wo, np.float32)
    bq = np.asarray(bq, np.float32)
    bk = np.asarray(bk, np.float32)
    bv = np.asarray(bv, np.float32)
    bo = np.asarray(bo, np.float32)

    nc = _build_nc()
    in_maps = make_in_maps(x, Wq, bq, Wk, bk, Wv, bv, Wo, bo)

    res = run_bass_kernel_spmd(nc, in_maps, list(range(8)), trace=TRACE,
                               **TRACE_KW)
    LAST_RESULTS = res

    # V bias folded: bo_eff = bo + (bv per head) @ Wo
    bv_heads = np.repeat(bv.reshape(G, DH), HPG, axis=0).reshape(-1)
    bo_eff = bo + bv_heads @ Wo

    y = np.empty((2, T, D), np.float32)
    for b in range(2):
        acc = res.results[b * G + 0]["yT"].astype(np.float32)
        for g in range(1, G):
            acc += res.results[b * G + g]["yT"].astype(np.float32)
        y[b] = acc.T + bo_eff
    return y


# revision 24
# speedup vs baseline: 1.0290x; 1.0290x over previous
"""GroupedQueryAttention Trainium2 Bass kernel (v2).

Sharding: 8 cores = (B=2) x (G=4 KV groups). Each core computes, for its
(batch b, kv-group g): the 4 query heads' Q/K/V projections, causal flash
attention, and a partial output projection Y^T_g (bf16). Host sums the 4
partials per batch and adds an adjusted bias (bo + bv-term folded in).

Key structure (all transposed: token dim T on the free axis):
  xT[d, t]     uploaded pre-transposed from host (bf16) - no PE transposes
  Q^T, K^T     from projection matmuls (W chunk stationary, xT moving)
  V^T -> V     PE transpose per 128-block, staged in the st PSUM slots
  S^T[s, t]  = (K^T s-block).T @ Q^T        (one 128-wide matmul per s-block)
  P^T        = exp(scale * S^T + mask)      (ACT, PSUM -> SBUF, bf16)
  O^T[dh, t] += (V s-block).T @ P^T         (PSUM accumulation over s-blocks)
  rowsum     += ones.T @ P^T                (PSUM accumulation, M=1)
  Y^T[dm, t] = sum_c (Wo chunk).T @ O^T_c   (per 128-row dm block, bf16 out)

The V bias never enters the kernel: O = (P@(V0+1*bv))/rowsum = P@V0/rowsum
+ bv, and the constant bv contribution to Y is folded into bo on the host.

oproj(tau-1) m-blocks are interleaved into flash(tau) as PE filler so the
S->exp->PV dependency chain's bubbles are absorbed by dense matmul work.

Normalize chain: rs copy (scalar) -> reciprocal_approx_fast [1,512] (DVE)
-> partition_broadcast (gpsimd) -> mul (DVE). Avoids the 3.3us exact
reciprocal.
"""

import sys

sys.path.insert(0, "/opt/trn_rl_repo")

from contextlib import ExitStack

import ml_dtypes
import numpy as np

import concourse.bass as bass  # noqa: F401
import concourse.tile as tile
from concourse import bacc, mybir
from concourse.bass_utils import run_bass_kernel_spmd

F32 = mybir.dt.float32
BF16 = mybir.dt.bfloat16
AF = mybir.ActivationFunctionType

D = 2048          # model dim
T = 2048          # tokens
DH = 128          # head dim
G = 4             # kv groups
HPG = 4           # query heads per group
QC = HPG * DH     # query cols per group = 512
ND = D // 128     # 16 contraction chunks
NTAU = 4          # t tiles of 512
TW = 512          # t tile width
SCALE = DH ** -0.5
NEG = -1e30

TRACE = False
TRACE_KW = {}
LAST_RESULTS = None

_CACHE = {}


def _body(ctx, tc, xT, wq, wk, wv, wo, bq, bk, maskTd, identd, yT):
    nc = tc.nc

    # PSUM: acc(2) + st(3, shared with V-transpose staging) + ot(2) + rs(1)
    psacc = ctx.enter_context(tc.tile_pool(name="psacc", bufs=2, space="PSUM"))
    psst = ctx.enter_context(tc.tile_pool(name="psst", bufs=3, space="PSUM"))
    psot = ctx.enter_context(tc.tile_pool(name="psot", bufs=2, space="PSUM"))
    psrs = ctx.enter_context(tc.tile_pool(name="psrs", bufs=1, space="PSUM"))

    consts = ctx.enter_context(tc.tile_pool(name="consts", bufs=1))
    qkv = ctx.enter_context(tc.tile_pool(name="qkv", bufs=1))
    xtp = ctx.enter_context(tc.tile_pool(name="xtp", bufs=ND))
    wkp = ctx.enter_context(tc.tile_pool(name="wkp", bufs=ND))
    wvp = ctx.enter_context(tc.tile_pool(name="wvp", bufs=ND))
    wqp = ctx.enter_context(tc.tile_pool(name="wqp", bufs=ND))
    wop = ctx.enter_context(tc.tile_pool(name="wop", bufs=1))
    vts = ctx.enter_context(tc.tile_pool(name="vstage", bufs=2))
    ptp = ctx.enter_context(tc.tile_pool(name="ptp", bufs=3))
    nrm = ctx.enter_context(tc.tile_pool(name="norm", bufs=2))
    otp_pool = ctx.enter_context(tc.tile_pool(name="otsb", bufs=1))
    yb = ctx.enter_context(tc.tile_pool(name="ybounce", bufs=3))

    # ---- constants on the scalar queue (small, early); wo also scalar
    # (needed late; the Activation HWDGE queue gets starved in arbitration
    # so nothing startup-critical goes on it)
    maskt = consts.tile([128, 128], F32, tag="maskt")
    nc.scalar.dma_start(maskt, maskTd)
    bqt = consts.tile([128, 4], F32, tag="bqt")
    nc.scalar.dma_start(bqt, bq.rearrange("(c p) -> p c", p=128))
    bkt = consts.tile([128, 1], F32, tag="bkt")
    nc.scalar.dma_start(bkt, bk.rearrange("(c p) -> p c", p=128))
    ident = consts.tile([128, 128], BF16, tag="ident")
    nc.scalar.dma_start(ident, identd)
    ones_col = consts.tile([128, 1], BF16, tag="ones_col")
    nc.vector.memset(ones_col, 1.0)

    # ---- weights + x on the two fast queues (sync HWDGE, gpsimd SWDGE),
    # strictly in first-use order: wk, x(sg0), wv, wq, x(sg1..3).
    xts = [xtp.tile([128, T], BF16, tag="xt", name=f"xt{d}") for d in range(ND)]
    wkts = [wkp.tile([128, DH], BF16, tag="wk", name=f"wk{d}") for d in range(ND)]
    wvts = [wvp.tile([128, DH], BF16, tag="wv", name=f"wv{d}") for d in range(ND)]
    wqts = [wqp.tile([128, QC], BF16, tag="wq", name=f"wq{d}") for d in range(ND)]
    wot = [wop.tile([128, D], BF16, tag=f"wo{c}", name=f"wo{c}") for c in range(HPG)]

    qlist = [nc.sync, nc.gpsimd]
    qi = 0

    def q_next():
        nonlocal qi
        eng = qlist[qi % 2]
        qi += 1
        return eng

    for d in range(ND):
        q_next().dma_start(wkts[d], wk[d * 128:(d + 1) * 128, :])
    for d in range(ND):
        q_next().dma_start(xts[d][:, 0:TW], xT[d * 128:(d + 1) * 128, 0:TW])
    for d in range(ND):
        q_next().dma_start(wvts[d], wv[d * 128:(d + 1) * 128, :])
    for d in range(ND):
        q_next().dma_start(wqts[d], wq[d * 128:(d + 1) * 128, :])
    for sg in range(1, NTAU):
        for d in range(ND):
            q_next().dma_start(
                xts[d][:, sg * TW:(sg + 1) * TW],
                xT[d * 128:(d + 1) * 128, sg * TW:(sg + 1) * TW])
    for c in range(HPG):
        nc.scalar.dma_start(wot[c], wo[c * 128:(c + 1) * 128, :])

    qt = [qkv.tile([128, T], BF16, tag=f"qt{j}", name=f"qt{j}") for j in range(HPG)]
    kt = qkv.tile([128, T], BF16, tag="kt")
    vv = qkv.tile([128, ND, 128], BF16, tag="vv")  # [s%128, s_block, dh]

    # ---- K + V projection for one sg column block
    def kvproj(sg):
        ps = psacc.tile([128, TW], F32, tag="acc", name="psk")
        for d in range(ND):
            nc.tensor.matmul(ps, wkts[d], xts[d][:, sg * TW:(sg + 1) * TW],
                             start=(d == 0), stop=(d == ND - 1))
        nc.scalar.activation(kt[:, sg * TW:(sg + 1) * TW], ps, AF.Identity,
                             bias=bkt[:, 0:1])

        ps2 = psacc.tile([128, TW], F32, tag="acc", name="psv")
        for d in range(ND):
            nc.tensor.matmul(ps2, wvts[d], xts[d][:, sg * TW:(sg + 1) * TW],
                             start=(d == 0), stop=(d == ND - 1))
        vtt = vts.tile([128, TW], BF16, tag="vt")
        nc.scalar.copy(vtt, ps2)
        # V^T -> V native, staged in an st-tag PSUM slot (idle until flash)
        stg = psst.tile([128, TW], BF16, tag="st", name="vstg")
        for i in range(4):
            nc.tensor.transpose(stg[:, i * 128:(i + 1) * 128],
                                vtt[:, i * 128:(i + 1) * 128], ident)
        nc.vector.tensor_copy(vv[:, sg * 4:(sg + 1) * 4, :], stg)

    # ---- Q projection for one t-tile (4 head blocks)
    def qproj(tau):
        for cb in range(HPG):
            ps = psacc.tile([128, TW], F32, tag="acc", name="psq")
            for d in range(ND):
                nc.tensor.matmul(
                    ps, wqts[d][:, cb * 128:(cb + 1) * 128],
                    xts[d][:, tau * TW:(tau + 1) * TW],
                    start=(d == 0), stop=(d == ND - 1))
            nc.scalar.activation(qt[cb][:, tau * TW:(tau + 1) * TW], ps,
                                 AF.Identity, bias=bqt[:, cb:cb + 1])

    # ---- output projection m-block (4 matmuls + copy + store)
    ots = [otp_pool.tile([128, T], BF16, tag=f"ot{j}", name=f"ots{j}")
           for j in range(HPG)]

    def oproj_block(tau, m):
        yp = psacc.tile([128, TW], F32, tag="acc", name="yp")
        for c in range(HPG):
            nc.tensor.matmul(
                yp, wot[c][:, m * 128:(m + 1) * 128],
                ots[c][:, tau * TW:(tau + 1) * TW],
                start=(c == 0), stop=(c == HPG - 1))
        ys = yb.tile([128, TW], BF16, tag="y", name="ys")
        if m % 2 == 0:
            nc.vector.tensor_copy(ys, yp)
        else:
            nc.scalar.copy(ys, yp)
        nc.sync.dma_start(
            yT[m * 128:(m + 1) * 128, tau * TW:(tau + 1) * TW], ys)

    # ---- phase C: per-tau pipeline [K,V,Q, flash(+oproj filler)]
    for tau in range(NTAU):
        kvproj(tau)
        qproj(tau)
        nsb = 4 * tau + 4
        # filler units: oproj(tau-1) m-blocks, spread over this tau's steps
        filler = []
        if tau > 0:
            filler = [(tau - 1, m) for m in range(ND)]
        total_steps = HPG * nsb
        fill_every = max(1, total_steps // max(len(filler), 1))
        step = 0
        fi = 0

        for j in range(HPG):
            otp = psot.tile([128, TW], F32, tag="ot")
            rs = psrs.tile([1, TW], F32, tag="rs")
            pts = {}
            qslice = qt[j][:, tau * TW:(tau + 1) * TW]

            def consume(sb, last):
                pt_, lo_ = pts[sb]
                nc.tensor.matmul(otp[:, lo_:], vv[:, sb, :], pt_[:, lo_:],
                                 start=(sb == 0), stop=last)
                nc.tensor.matmul(rs[:, lo_:], ones_col, pt_[:, lo_:],
                                 start=(sb == 0), stop=last)

            for sb in range(nsb):
                di = sb - 4 * tau
                lo = di * 128 if di >= 0 else 0   # valid t-range start
                st = psst.tile([128, TW], F32, tag="st")
                nc.tensor.matmul(st[:, lo:], kt[:, sb * 128:(sb + 1) * 128],
                                 qslice[:, lo:], start=True, stop=True)
                if di >= 0:  # triangle mask on the first 128 valid columns
                    nc.vector.tensor_add(st[:, lo:lo + 128], st[:, lo:lo + 128],
                                         maskt)
                pt = ptp.tile([128, TW], BF16, tag="pt")
                nc.scalar.activation(pt[:, lo:], st[:, lo:], AF.Exp, scale=SCALE)
                pts[sb] = (pt, lo)
                # software-pipeline PE: PV/rowsum issue 2 s-blocks behind
                if sb >= 2:
                    consume(sb - 2, last=(sb - 2 == nsb - 1))
                    del pts[sb - 2]
                step += 1
                if fi < len(filler) and step % fill_every == 0:
                    oproj_block(*filler[fi])
                    fi += 1
            for sb in (nsb - 2, nsb - 1):
                if sb >= 0 and sb in pts:
                    consume(sb, last=(sb == nsb - 1))

            # normalize: O^T * (1/rowsum)
            rsb = nrm.tile([1, TW], F32, tag="rsb")
            nc.scalar.copy(rsb, rs)
            rc1 = nrm.tile([1, TW], F32, tag="rc1")
            nc.vector.reciprocal_approx_fast(rc1, rsb)
            rc = nrm.tile([128, TW], F32, tag="rc")
            nc.gpsimd.partition_broadcast(rc, rc1)
            nc.vector.tensor_mul(ots[j][:, tau * TW:(tau + 1) * TW], otp, rc)

        while fi < len(filler):
            oproj_block(*filler[fi])
            fi += 1
    for m in range(ND):
        oproj_block(NTAU - 1, m)


def _build_nc():
    if "nc" in _CACHE:
        return _CACHE["nc"]
    nc = bacc.Bacc("TRN2", target_bir_lowering=False, debug=False)
    xT = nc.dram_tensor("xT", [D, T], BF16, kind="ExternalInput").ap()
    wq = nc.dram_tensor("wq", [D, QC], BF16, kind="ExternalInput").ap()
    wk = nc.dram_tensor("wk", [D, DH], BF16, kind="ExternalInput").ap()
    wv = nc.dram_tensor("wv", [D, DH], BF16, kind="ExternalInput").ap()
    wo = nc.dram_tensor("wo", [QC, D], BF16, kind="ExternalInput").ap()
    bq = nc.dram_tensor("bq", [QC], F32, kind="ExternalInput").ap()
    bk = nc.dram_tensor("bk", [DH], F32, kind="ExternalInput").ap()
    maskTd = nc.dram_tensor("maskT", [128, 128], F32, kind="ExternalInput").ap()
    identd = nc.dram_tensor("ident", [128, 128], BF16, kind="ExternalInput").ap()
    yT = nc.dram_tensor("yT", [D, T], BF16, kind="ExternalOutput").ap()

    with tile.TileContext(nc) as tc, ExitStack() as ctx:
        _body(ctx, tc, xT, wq, wk, wv, wo, bq, bk, maskTd, identd, yT)
    nc.compile()
    _CACHE["nc"] = nc
    return nc


def _host_consts():
    p = np.arange(128)[:, None]
    f = np.arange(128)[None, :]
    maskT = np.where(f >= p, 0.0, NEG).astype(np.float32)
    ident = np.eye(128, dtype=ml_dtypes.bfloat16)
    return maskT, ident


def make_in_maps(x, Wq, bq, Wk, bk, Wv, bv, Wo, bo):
    maskT, ident = _host_consts()
    bf = lambda a: np.ascontiguousarray(a).astype(ml_dtypes.bfloat16)

    xTb = [bf(x[b].T) for b in range(2)]
    in_maps = []
    for c in range(8):
        b, g = divmod(c, G)
        in_maps.append({
            "xT": xTb[b],
            "wq": bf(Wq[:, g * QC:(g + 1) * QC]),
            "wk": bf(Wk[:, g * DH:(g + 1) * DH]),
            "wv": bf(Wv[:, g * DH:(g + 1) * DH]),
            "wo": bf(Wo[g * QC:(g + 1) * QC, :]),
            "bq": np.ascontiguousarray(bq[g * QC:(g + 1) * QC]),
            "bk": np.ascontiguousarray(bk[g * DH:(g + 1) * DH]),
            "maskT": maskT,
            "ident": ident,
        })
    return in_maps


def kernel(x, Wq, bq, Wk, bk, Wv, bv, Wo, bo):
    global LAST_RESULTS
    x = np.asarray(x, np.float32)
    Wq = np.asarray(Wq, np.float32)
    Wk = np.asarray(Wk, np.float32)
    Wv = np.asarray(Wv, np.float32)
    Wo = np.asarray(Wo, np.float32)
    bq = np.asarray(bq, np.float32)
    bk = np.asarray(bk, np.float32)
    bv = np.asarray(bv, np.float32)
    bo = np.asarray(bo, np.float32)

    nc = _build_nc()
    in_maps = make_in_maps(x, Wq, bq, Wk, bk, Wv, bv, Wo, bo)

    res = run_bass_kernel_spmd(nc, in_maps, list(range(8)), trace=TRACE,
                               **TRACE_KW)
    LAST_RESULTS = res

    # V bias folded: bo_eff = bo + (bv per head) @ Wo
    bv_heads = np.repeat(bv.reshape(G, DH), HPG, axis=0).reshape(-1)
    bo_eff = bo + bv_heads @ Wo

    y = np.empty((2, T, D), np.float32)
    for b in range(2):
        acc = res.results[b * G + 0]["yT"].astype(np.float32)
        for g in range(1, G):
            acc += res.results[b * G + g]["yT"].astype(np.float32)
        y[b] = acc.T + bo_eff
    return y


# revision 26
# speedup vs baseline: 1.0440x; 1.0146x over previous
"""GroupedQueryAttention Trainium2 Bass kernel (v2).

Sharding: 8 cores = (B=2) x (G=4 KV groups). Each core computes, for its
(batch b, kv-group g): the 4 query heads' Q/K/V projections, causal flash
attention, and a partial output projection Y^T_g (bf16). Host sums the 4
partials per batch and adds an adjusted bias (bo + bv-term folded in).

Key structure (all transposed: token dim T on the free axis):
  xT[d, t]     uploaded pre-transposed from host (bf16) - no PE transposes
  Q^T, K^T     from projection matmuls (W chunk stationary, xT moving)
  V^T -> V     PE transpose per 128-block, staged in the st PSUM slots
  S^T[s, t]  = (K^T s-block).T @ Q^T        (one 128-wide matmul per s-block)
  P^T        = exp(scale * S^T + mask)      (ACT, PSUM -> SBUF, bf16)
  O^T[dh, t] += (V s-block).T @ P^T         (PSUM accumulation over s-blocks)
  rowsum     += ones.T @ P^T                (PSUM accumulation, M=1)
  Y^T[dm, t] = sum_c (Wo chunk).T @ O^T_c   (per 128-row dm block, bf16 out)

The V bias never enters the kernel: O = (P@(V0+1*bv))/rowsum = P@V0/rowsum
+ bv, and the constant bv contribution to Y is folded into bo on the host.

oproj(tau-1) m-blocks are interleaved into flash(tau) as PE filler so the
S->exp->PV dependency chain's bubbles are absorbed by dense matmul work.

Normalize chain: rs copy (scalar) -> reciprocal_approx_fast [1,512] (DVE)
-> partition_broadcast (gpsimd) -> mul (DVE). Avoids the 3.3us exact
reciprocal.
"""

import sys

sys.path.insert(0, "/opt/trn_rl_repo")

from contextlib import ExitStack

import ml_dtypes
import numpy as np

import concourse.bass as bass  # noqa: F401
import concourse.tile as tile
from concourse import bacc, mybir
from concourse.bass_utils import run_bass_kernel_spmd

F32 = mybir.dt.float32
BF16 = mybir.dt.bfloat16
AF = mybir.ActivationFunctionType

D = 2048          # model dim
T = 2048          # tokens
DH = 128          # head dim
G = 4             # kv groups
HPG = 4           # query heads per group
QC = HPG * DH     # query cols per group = 512
ND = D // 128     # 16 contraction chunks
NTAU = 4          # t tiles of 512
TW = 512          # t tile width
SCALE = DH ** -0.5
NEG = -1e30

TRACE = False
TRACE_KW = {}
LAST_RESULTS = None

_CACHE = {}


def _body(ctx, tc, xT, wq, wk, wv, wo, bq, bk, maskTd, identd, yT):
    nc = tc.nc

    # PSUM: acc(2) + st(3, shared with V-transpose staging) + ot(2) + rs(1)
    psacc = ctx.enter_context(tc.tile_pool(name="psacc", bufs=2, space="PSUM"))
    psst = ctx.enter_context(tc.tile_pool(name="psst", bufs=3, space="PSUM"))
    psot = ctx.enter_context(tc.tile_pool(name="psot", bufs=2, space="PSUM"))
    psrs = ctx.enter_context(tc.tile_pool(name="psrs", bufs=1, space="PSUM"))

    consts = ctx.enter_context(tc.tile_pool(name="consts", bufs=1))
    qkv = ctx.enter_context(tc.tile_pool(name="qkv", bufs=1))
    xtp = ctx.enter_context(tc.tile_pool(name="xtp", bufs=ND))
    wkp = ctx.enter_context(tc.tile_pool(name="wkp", bufs=ND))
    wvp = ctx.enter_context(tc.tile_pool(name="wvp", bufs=ND))
    wqp = ctx.enter_context(tc.tile_pool(name="wqp", bufs=ND))
    wop = ctx.enter_context(tc.tile_pool(name="wop", bufs=1))
    vts = ctx.enter_context(tc.tile_pool(name="vstage", bufs=2))
    ptp = ctx.enter_context(tc.tile_pool(name="ptp", bufs=4))
    nrm = ctx.enter_context(tc.tile_pool(name="norm", bufs=2))
    otp_pool = ctx.enter_context(tc.tile_pool(name="otsb", bufs=1))
    yb = ctx.enter_context(tc.tile_pool(name="ybounce", bufs=3))

    # ---- constants on the scalar queue (small, early); wo also scalar
    # (needed late; the Activation HWDGE queue gets starved in arbitration
    # so nothing startup-critical goes on it)
    maskt = consts.tile([128, 128], F32, tag="maskt")
    nc.scalar.dma_start(maskt, maskTd)
    bqt = consts.tile([128, 4], F32, tag="bqt")
    nc.scalar.dma_start(bqt, bq.rearrange("(c p) -> p c", p=128))
    bkt = consts.tile([128, 1], F32, tag="bkt")
    nc.scalar.dma_start(bkt, bk.rearrange("(c p) -> p c", p=128))
    ident = consts.tile([128, 128], BF16, tag="ident")
    nc.scalar.dma_start(ident, identd)
    ones_col = consts.tile([128, 1], BF16, tag="ones_col")
    nc.vector.memset(ones_col, 1.0)

    # ---- weights + x on the two fast queues (sync HWDGE, gpsimd SWDGE),
    # strictly in first-use order: wk, x(sg0), wv, wq, x(sg1..3).
    xts = [xtp.tile([128, T], BF16, tag="xt", name=f"xt{d}") for d in range(ND)]
    wkts = [wkp.tile([128, DH], BF16, tag="wk", name=f"wk{d}") for d in range(ND)]
    wvts = [wvp.tile([128, DH], BF16, tag="wv", name=f"wv{d}") for d in range(ND)]
    wqts = [wqp.tile([128, QC], BF16, tag="wq", name=f"wq{d}") for d in range(ND)]
    wot = [wop.tile([128, D], BF16, tag=f"wo{c}", name=f"wo{c}") for c in range(HPG)]

    qlist = [nc.sync, nc.gpsimd]
    qi = 0

    def q_next():
        nonlocal qi
        eng = qlist[qi % 2]
        qi += 1
        return eng

    for d in range(ND):
        q_next().dma_start(wkts[d], wk[d * 128:(d + 1) * 128, :])
    for d in range(ND):
        q_next().dma_start(xts[d][:, 0:TW], xT[d * 128:(d + 1) * 128, 0:TW])
    for d in range(ND):
        q_next().dma_start(wvts[d], wv[d * 128:(d + 1) * 128, :])
    for d in range(ND):
        q_next().dma_start(wqts[d], wq[d * 128:(d + 1) * 128, :])
    for sg in range(1, NTAU):
        for d in range(ND):
            q_next().dma_start(
                xts[d][:, sg * TW:(sg + 1) * TW],
                xT[d * 128:(d + 1) * 128, sg * TW:(sg + 1) * TW])
    for c in range(HPG):
        nc.scalar.dma_start(wot[c], wo[c * 128:(c + 1) * 128, :])

    # ---- HAM warm-up: real matmuls on ident while the x DMAs land, so the
    # PE clock-gate is already at 8/8 when the K projection starts (PE-mode
    # transposes don't count as HAM activity; these do)
    for w in range(56):
        wps = psot.tile([128, 128], F32, tag="ot", name="warm")
        nc.tensor.matmul(wps, ident, ident, start=True, stop=True)

    qt = [qkv.tile([128, T], BF16, tag=f"qt{j}", name=f"qt{j}") for j in range(HPG)]
    kt = qkv.tile([128, T], BF16, tag="kt")
    vv = qkv.tile([128, ND, 128], BF16, tag="vv")  # [s%128, s_block, dh]

    # ---- K + V projection for one sg column block
    def kvproj(sg):
        ps = psacc.tile([128, TW], F32, tag="acc", name="psk")
        for d in range(ND):
            nc.tensor.matmul(ps, wkts[d], xts[d][:, sg * TW:(sg + 1) * TW],
                             start=(d == 0), stop=(d == ND - 1))
        nc.scalar.activation(kt[:, sg * TW:(sg + 1) * TW], ps, AF.Identity,
                             bias=bkt[:, 0:1])

        ps2 = psacc.tile([128, TW], F32, tag="acc", name="psv")
        for d in range(ND):
            nc.tensor.matmul(ps2, wvts[d], xts[d][:, sg * TW:(sg + 1) * TW],
                             start=(d == 0), stop=(d == ND - 1))
        vtt = vts.tile([128, TW], BF16, tag="vt")
        nc.scalar.copy(vtt, ps2)
        # V^T -> V native, staged in an st-tag PSUM slot (idle until flash)
        stg = psst.tile([128, TW], BF16, tag="st", name="vstg")
        for i in range(4):
            nc.tensor.transpose(stg[:, i * 128:(i + 1) * 128],
                                vtt[:, i * 128:(i + 1) * 128], ident)
        nc.vector.tensor_copy(vv[:, sg * 4:(sg + 1) * 4, :], stg)

    # ---- Q projection for one t-tile (4 head blocks)
    def qproj(tau):
        for cb in range(HPG):
            ps = psacc.tile([128, TW], F32, tag="acc", name="psq")
            for d in range(ND):
                nc.tensor.matmul(
                    ps, wqts[d][:, cb * 128:(cb + 1) * 128],
                    xts[d][:, tau * TW:(tau + 1) * TW],
                    start=(d == 0), stop=(d == ND - 1))
            nc.scalar.activation(qt[cb][:, tau * TW:(tau + 1) * TW], ps,
                                 AF.Identity, bias=bqt[:, cb:cb + 1])

    # ---- output projection m-block (4 matmuls + copy + store)
    ots = [otp_pool.tile([128, T], BF16, tag=f"ot{j}", name=f"ots{j}")
           for j in range(HPG)]

    def oproj_block(tau, m):
        yp = psacc.tile([128, TW], F32, tag="acc", name="yp")
        for c in range(HPG):
            nc.tensor.matmul(
                yp, wot[c][:, m * 128:(m + 1) * 128],
                ots[c][:, tau * TW:(tau + 1) * TW],
                start=(c == 0), stop=(c == HPG - 1))
        ys = yb.tile([128, TW], BF16, tag="y", name="ys")
        if m % 2 == 0:
            nc.vector.tensor_copy(ys, yp)
        else:
            nc.scalar.copy(ys, yp)
        nc.sync.dma_start(
            yT[m * 128:(m + 1) * 128, tau * TW:(tau + 1) * TW], ys)

    # ---- phase C: per-tau pipeline [K,V,Q, flash(+oproj filler)]
    for tau in range(NTAU):
        kvproj(tau)
        qproj(tau)
        nsb = 4 * tau + 4
        # filler units: oproj(tau-1) m-blocks, spread over this tau's steps
        filler = []
        if tau > 0:
            filler = [(tau - 1, m) for m in range(ND)]
        total_steps = HPG * nsb
        fill_every = max(1, total_steps // max(len(filler), 1))
        step = 0
        fi = 0

        for j in range(HPG):
            otp = psot.tile([128, TW], F32, tag="ot")
            rs = psrs.tile([1, TW], F32, tag="rs")
            pts = {}
            qslice = qt[j][:, tau * TW:(tau + 1) * TW]

            def consume(sb, last):
                pt_, lo_ = pts[sb]
                nc.tensor.matmul(otp[:, lo_:], vv[:, sb, :], pt_[:, lo_:],
                                 start=(sb == 0), stop=last)
                nc.tensor.matmul(rs[:, lo_:], ones_col, pt_[:, lo_:],
                                 start=(sb == 0), stop=last)

            for sb in range(nsb):
                di = sb - 4 * tau
                lo = di * 128 if di >= 0 else 0   # valid t-range start
                st = psst.tile([128, TW], F32, tag="st")
                nc.tensor.matmul(st[:, lo:], kt[:, sb * 128:(sb + 1) * 128],
                                 qslice[:, lo:], start=True, stop=True)
                if di >= 0:  # triangle mask on the first 128 valid columns
                    nc.vector.tensor_add(st[:, lo:lo + 128], st[:, lo:lo + 128],
                                         maskt)
                pt = ptp.tile([128, TW], BF16, tag="pt")
                nc.scalar.activation(pt[:, lo:], st[:, lo:], AF.Exp, scale=SCALE)
                pts[sb] = (pt, lo)
                # software-pipeline PE: PV/rowsum issue 2 s-blocks behind
                if sb >= 2:
                    consume(sb - 2, last=(sb - 2 == nsb - 1))
                    del pts[sb - 2]
                step += 1
                if fi < len(filler) and step % fill_every == 0:
                    oproj_block(*filler[fi])
                    fi += 1
            for sb in (nsb - 2, nsb - 1):
                if sb >= 0 and sb in pts:
                    consume(sb, last=(sb == nsb - 1))

            # normalize: O^T * (1/rowsum)
            rsb = nrm.tile([1, TW], F32, tag="rsb")
            nc.scalar.copy(rsb, rs)
            rc1 = nrm.tile([1, TW], F32, tag="rc1")
            nc.vector.reciprocal_approx_fast(rc1, rsb)
            rc = nrm.tile([128, TW], F32, tag="rc")
            nc.gpsimd.partition_broadcast(rc, rc1)
            nc.vector.tensor_mul(ots[j][:, tau * TW:(tau + 1) * TW], otp, rc)

        while fi < len(filler):
            oproj_block(*filler[fi])
            fi += 1
    for m in range(ND):
        oproj_block(NTAU - 1, m)


def _build_nc():
    if "nc" in _CACHE:
        return _CACHE["nc"]
    nc = bacc.Bacc("TRN2", target_bir_lowering=False, debug=False)
    xT = nc.dram_tensor("xT", [D, T], BF16, kind="ExternalInput").ap()
    wq = nc.dram_tensor("wq", [D, QC], BF16, kind="ExternalInput").ap()
    wk = nc.dram_tensor("wk", [D, DH], BF16, kind="ExternalInput").ap()
    wv = nc.dram_tensor("wv", [D, DH], BF16, kind="ExternalInput").ap()
    wo = nc.dram_tensor("wo", [QC, D], BF16, kind="ExternalInput").ap()
    bq = nc.dram_tensor("bq", [QC], F32, kind="ExternalInput").ap()
    bk = nc.dram_tensor("bk", [DH], F32, kind="ExternalInput").ap()
    maskTd = nc.dram_tensor("maskT", [128, 128], F32, kind="ExternalInput").ap()
    identd = nc.dram_tensor("ident", [128, 128], BF16, kind="ExternalInput").ap()
    yT = nc.dram_tensor("yT", [D, T], BF16, kind="ExternalOutput").ap()

    with tile.TileContext(nc) as tc, ExitStack() as ctx:
        _body(ctx, tc, xT, wq, wk, wv, wo, bq, bk, maskTd, identd, yT)
    nc.compile()
    _CACHE["nc"] = nc
    return nc


def _host_consts():
    p = np.arange(128)[:, None]
    f = np.arange(128)[None, :]
    maskT = np.where(f >= p, 0.0, NEG).astype(np.float32)
    ident = np.eye(128, dtype=ml_dtypes.bfloat16)
    return maskT, ident


def make_in_maps(x, Wq, bq, Wk, bk, Wv, bv, Wo, bo):
    maskT, ident = _host_consts()
    bf = lambda a: np.ascontiguousarray(a).astype(ml_dtypes.bfloat16)

    xTb = [bf(x[b].T) for b in range(2)]
    in_maps = []
    for c in range(8):
        b, g = divmod(c, G)
        in_maps.append({
            "xT": xTb[b],
            "wq": bf(Wq[:, g * QC:(g + 1) * QC]),
            "wk": bf(Wk[:, g * DH:(g + 1) * DH]),
            "wv": bf(Wv[:, g * DH:(g + 1) * DH]),
            "wo": bf(Wo[g * QC:(g + 1) * QC, :]),
            "bq": np.ascontiguousarray(bq[g * QC:(g + 1) * QC]),
            "bk": np.ascontiguousarray(bk[g * DH:(g + 1) * DH]),
            "maskT": maskT,
            "ident": ident,
        })
    return in_maps


def kernel(x, Wq, bq, Wk, bk, Wv, bv, Wo, bo):
    global LAST_RESULTS
    x = np.asarray(x, np.float32)
    Wq = np.asarray(Wq, np.float32)
    Wk = np.asarray(Wk, np.float32)
    Wv = np.asarray(Wv, np.float32)
    Wo = np.asarray(Wo, np.float32)
    bq = np.asarray(bq, np.float32)
    bk = np.asarray(bk, np.float32)
    bv = np.asarray(bv, np.float32)
    bo = np.asarray(bo, np.float32)

    nc = _build_nc()
    in_maps = make_in_maps(x, Wq, bq, Wk, bk, Wv, bv, Wo, bo)

    res = run_bass_kernel_spmd(nc, in_maps, list(range(8)), trace=TRACE,
                               **TRACE_KW)
    LAST_RESULTS = res

    # V bias folded: bo_eff = bo + (bv per head) @ Wo
    bv_heads = np.repeat(bv.reshape(G, DH), HPG, axis=0).reshape(-1)
    bo_eff = bo + bv_heads @ Wo

    y = np.empty((2, T, D), np.float32)
    for b in range(2):
        acc = res.results[b * G + 0]["yT"].astype(np.float32)
        for g in range(1, G):
            acc += res.results[b * G + g]["yT"].astype(np.float32)
        y[b] = acc.T + bo_eff
    return y
